# revision 2
# baseline (speedup 1.0000x reference)
"""Trainium2 Bass kernel for nn_DirectionalProcessor — v2.

Same folded-conv math as v1: out[p] = sum_d x[p - delta_d] @ M_d + bc with
M_d = Wd[d] @ Wc[:, d*C:(d+1)*C].T folded on device.  Padded-position
tiling as v1: tile j = 128 consecutive flat padded positions, so every
shifted tap window is a contiguous 1-D slice (walrus requires a single
free dim on the stationary matmul operand).

v2 changes vs v1:
  - x host-cast to fp16 (same rounding the v1 device cast-DMA applied);
    all input DMAs move from SWDGE to HWDGE on the SP queue.
  - fp16 output (adds ~2e-4 rel err, budget 2e-2), partition-major HBM
    layout [img, p, tile, oc] so each output DMA descriptor is a 1 KiB
    contiguous run; host untangles for free.
  - per-pair output DMAs to shrink the drain tail.
  - fold interleaved with early conv passes: weight chunks stream in
    direction pairs; as soon as a pair's M_d is folded, its taps run over
    the first 3 psum pairs (tiles 0..5, inside the head strip of image 0),
    so the PE never waits on the weight-DMA tail.
"""

import numpy as np

import concourse.bass as bass
import concourse.bacc as bacc
import concourse.mybir as mybir
import concourse.tile as tile
from concourse.bass_utils import run_bass_kernel_spmd

B, H, W, C = 16, 64, 64, 256
DIRECTIONS = [(0, -1), (1, -1), (1, 0), (1, 1), (0, 1), (-1, 1), (-1, 0), (-1, -1)]
N_CORES = 8
BPC = B // N_CORES  # images per core
HP = H + 2  # 66: padded spatial extent
XF = HP * HP + 2  # 4358: flat padded image + sentinel zero at each end
NQ = H * HP  # 4224: padded output positions per image
NT = (NQ + 127) // 128  # 33 output tiles per image
F16 = mybir.dt.float16
F32 = mybir.dt.float32

LAST_RESULTS = None


def build_bass() -> bass.Bass:
    nc = bacc.Bacc(None)

    xp_d = nc.dram_tensor("xp", [BPC, C, XF], F16, kind="ExternalInput")
    wdt_d = nc.dram_tensor("wdt", [128, 8, 2, C], F16, kind="ExternalInput")
    wct_d = nc.dram_tensor("wct", [128, 8, 2, C], F16, kind="ExternalInput")
    b_d = nc.dram_tensor("bias", [1, 512], F16, kind="ExternalInput")
    # partition-major output: [img, p, tile, oc]; host untangles
    out_d = nc.dram_tensor("out", [BPC, 128, NT, C], F16, kind="ExternalOutput")

    deltas = [-(dy * HP + dx) for (dx, dy) in DIRECTIONS]

    with tile.TileContext(nc) as tc:
        with (
            tc.tile_pool(name="const", bufs=1) as const,
            tc.tile_pool(name="foldps", bufs=2, space="PSUM") as fold_pool,
            tc.tile_pool(name="mainps", bufs=3, space="PSUM") as psum_pool,
            tc.tile_pool(name="warmps", bufs=1, space="PSUM") as warm_pool,
            tc.tile_pool(name="osb", bufs=4) as osb_pool,
        ):
            # ---- PE pre-warm: keep PE busy from t~0.2us so the pstate ramp
            # reaches full speed by the time the fold starts. Uninitialized
            # SBUF is fine — the warm psum is never read ----
            warm16 = const.tile([128, 256], F16, tag="warm16")
            nc.vector.memset(warm16[0:1, 0:1], 0.0)  # allocate; rest garbage
            wps = warm_pool.tile([128, 256], F32, tag="warm")
            for _ in range(8):
                nc.tensor.matmul(wps[:], lhsT=warm16[:, 0:128], rhs=warm16[:])

            wdt32 = const.tile([128, 8, 2, C], F16, tag="wdt32")
            wct32 = const.tile([128, 8, 2, C], F16, tag="wct32")
            bias16 = const.tile([1, 512], F16, tag="bias16")
            xts = []
            for img in range(BPC):
                per = []
                for ch in range(2):
                    t = const.tile(
                        [128, XF], F16, tag=f"xp_{img}_{ch}", name=f"xp_{img}_{ch}"
                    )
                    per.append(t)
                xts.append(per)
            S0 = 928  # head strip: flat cols 0..927 (rows 0..13), tiles 0..5
            S1 = 2674  # second slab boundary (rows ..40)

            nc.sync.dma_start(out=wdt32[:, 0:2], in_=wdt_d[:][:, 0:2])
            nc.sync.dma_start(out=wct32[:, 0:2], in_=wct_d[:][:, 0:2])
            for ch in range(2):  # img0 head strip
                nc.sync.dma_start(
                    out=xts[0][ch][:, 0:S0],
                    in_=xp_d[:][0, ch * 128 : (ch + 1) * 128, 0:S0],
                )
            nc.sync.dma_start(out=wdt32[:, 2:4], in_=wdt_d[:][:, 2:4])
            nc.sync.dma_start(out=wct32[:, 2:4], in_=wct_d[:][:, 2:4])
            nc.sync.dma_start(out=wdt32[:, 4:6], in_=wdt_d[:][:, 4:6])
            nc.sync.dma_start(out=wct32[:, 4:6], in_=wct_d[:][:, 4:6])
            nc.sync.dma_start(out=wdt32[:, 6:8], in_=wdt_d[:][:, 6:8])
            nc.sync.dma_start(out=wct32[:, 6:8], in_=wct_d[:][:, 6:8])
            nc.sync.dma_start(out=bias16[:], in_=b_d[:])
            for ch in range(2):  # img0 remainder in two slabs
                nc.sync.dma_start(
                    out=xts[0][ch][:, S0:S1],
                    in_=xp_d[:][0, ch * 128 : (ch + 1) * 128, S0:S1],
                )
            for ch in range(2):
                nc.sync.dma_start(
                    out=xts[0][ch][:, S1:XF],
                    in_=xp_d[:][0, ch * 128 : (ch + 1) * 128, S1:XF],
                )
            for ch in range(2):  # img1 whole
                nc.sync.dma_start(
                    out=xts[1][ch][:],
                    in_=xp_d[:][1, ch * 128 : (ch + 1) * 128, :],
                )

            # ---- fold + early conv, interleaved with the weight stream ----
            m16 = const.tile([128, 2, 8, C], F16, tag="m16")

            def fold_pair(dlo):
                for d in (dlo, dlo + 1):
                    mp = fold_pool.tile([128, 512], F32, tag="fps", name=f"mdps_{d}")
                    for cc in range(2):
                        for ec in range(2):
                            nc.tensor.matmul(
                                mp[:, cc * 256 : (cc + 1) * 256],
                                lhsT=wdt32[:, d, ec, cc * 128 : (cc + 1) * 128],
                                rhs=wct32[:, d, ec, :],
                                start=(ec == 0),
                                stop=(ec == 1),
                                skip_group_check=True,
                            )
                    nc.vector.tensor_copy(m16[:, :, d, :], mp[:])

            def emit_taps(pt, img, tiles, dis):
                # Each call is a CLOSED accumulation group per psum region
                # (start only on the very first tap, stop at the end of every
                # call): interleaved open groups corrupt the first region on
                # real HW. stop is a no-op on HW, so accumulation carries
                # across calls via start=False.
                x0, x1 = xts[img][0], xts[img][1]
                for t, j in enumerate(tiles):
                    for di in dis:
                        s = 67 + 128 * j + deltas[di]
                        for ch, xt in enumerate((x0, x1)):
                            nc.tensor.matmul(
                                pt[:, t * 256 : (t + 1) * 256],
                                lhsT=xt[:, s : s + 128],
                                rhs=m16[:, ch, di, :],
                                start=(di == 0 and ch == 0),
                                stop=(di == dis[-1] and ch == 1),
                                skip_group_check=True,
                            )

            def emit_evac(pt, img, tiles):
                n = len(tiles)
                ot = osb_pool.tile(
                    [128, n, 256], F16, tag="osb", name=f"ot{img}_{tiles[0]}"
                )
                nc.vector.tensor_add(
                    ot[:].rearrange("p t o -> p (t o)"),
                    pt[:, : n * 256],
                    bias_sb[:, : n * 256],
                )
                nc.sync.dma_start(
                    out=out_d[:][img, :, tiles[0] : tiles[0] + n, :], in_=ot[:]
                )

            # Sequential fold: interleaving open accumulation groups with the
            # early conv corrupts the first psum region per pair on real HW
            # (verified empirically), so fold all directions first.
            NEARLY = 0
            fold_pair(0)
            fold_pair(2)
            fold_pair(4)
            fold_pair(6)

            # ---- bias broadcast to [128, 512] f32 via rank-1 matmul ----
            bp = fold_pool.tile([128, 512], F32, tag="fps", name="biasps")
            nc.tensor.matmul(bp[:, 0:256], lhsT=bias16[:, 0:128], rhs=bias16[:, 256:512])
            nc.tensor.matmul(bp[:, 256:512], lhsT=bias16[:, 0:128], rhs=bias16[:, 256:512])
            bias_sb = const.tile([128, 512], F32, tag="bias_sb")
            nc.vector.tensor_copy(bias_sb[:], bp[:])

            # ---- main conv loop over all tile pairs ----
            for img in range(BPC):
                start_jp = NEARLY if img == 0 else 0
                for jp in range(start_jp, (NT + 1) // 2):
                    tiles = [j for j in (2 * jp, 2 * jp + 1) if j < NT]
                    pt = psum_pool.tile(
                        [128, 512], F32, tag="ps", name=f"ps{img}_{jp}"
                    )
                    emit_taps(pt, img, tiles, list(range(8)))
                    emit_evac(pt, img, tiles)

    nc.finalize()
    return nc


def _host_prep(grid_embedding, Wd, Wc, bc):
    g = np.asarray(grid_embedding, dtype=np.float32)
    gpad = np.zeros((B, C, XF), np.float16)
    gview = gpad[:, :, 1 : 1 + HP * HP].reshape(B, C, HP, HP)
    gview[:, :, 1 : H + 1, 1 : W + 1] = g.transpose(0, 3, 1, 2)
    wdt_dec = np.asarray(Wd, np.float32).transpose(0, 2, 1)
    wct_dec = np.asarray(Wc, np.float32).reshape(C, 8, C).transpose(1, 2, 0)
    wdt = np.ascontiguousarray(
        wdt_dec.reshape(8, 2, 128, C).transpose(2, 0, 1, 3).astype(np.float16)
    )
    wct = np.ascontiguousarray(
        wct_dec.reshape(8, 2, 128, C).transpose(2, 0, 1, 3).astype(np.float16)
    )
    bias = np.zeros((1, 512), np.float16)
    bias[0, :128] = 1.0
    bias[0, 256:512] = np.asarray(bc, np.float16)
    return gpad, wdt, wct, bias


def _untangle_out(arr):
    # [BPC, 128, NT, C] -> flat padded positions 128*j + p -> unpad
    flat = arr.transpose(0, 2, 1, 3).reshape(BPC, NT * 128, C)
    o = flat[:, :NQ].reshape(BPC, H, HP, C)
    return o[:, :, 1 : W + 1, :].astype(np.float32)


_NC_CACHE = {}


def kernel(grid_embedding, Wd, Wc, bc):
    global LAST_RESULTS
    gpad, wdt, wct, bias = _host_prep(grid_embedding, Wd, Wc, bc)

    if "nc" not in _NC_CACHE:
        _NC_CACHE["nc"] = build_bass()
    nc = _NC_CACHE["nc"]

    in_maps = [
        {
            "xp": np.ascontiguousarray(gpad[core * BPC : (core + 1) * BPC]),
            "wdt": wdt,
            "wct": wct,
            "bias": bias,
        }
        for core in range(N_CORES)
    ]
    res = run_bass_kernel_spmd(nc, in_maps, core_ids=list(range(N_CORES)))
    LAST_RESULTS = res
    out = np.concatenate([_untangle_out(r["out"]) for r in res.results], axis=0)
    return np.ascontiguousarray(out)


if __name__ == "__main__":
    rng = np.random.default_rng(0)
    inputs = {
        "grid_embedding": rng.standard_normal((B, H, W, C), dtype=np.float32),
        "Wd": (rng.standard_normal((8, C, C)) * 0.01).astype(np.float32),
        "Wc": (rng.standard_normal((C, 8 * C)) * 0.02).astype(np.float32),
        "bc": (rng.standard_normal(C) * 0.02).astype(np.float32),
    }
    out = kernel(**inputs)
    print("out", out.shape, out.dtype)
